# revision 19
# baseline (speedup 1.0000x reference)
"""CTAN (gnn_message_passing) Trainium2 kernel — 8 NeuronCores, edge-parallel.

v2: alpha/softmax moved off DVE onto PE/ACT.
  - Host: shard nodes into 8 contiguous ranges balanced by in-degree; edges go
    to the core owning their dst, windowed by dst into 128-node windows,
    lo/hi src-row split for int16 dma_gather, padded to 128-edge chunks.
  - Node phase: per window, x -> q^T (kept in SBUF, bf16), k|v (bf16, DRAM,
    AllGather), xa = x@A.T.
  - Edge phase per chunk: dma_gather kv[src] rows; PE-transpose k; kse^T =
    k^T + e^T (DVE); AL[e,n] = kse^T^T @ q^T_win (PE); pv = exp(scale*AL)
    (ACT); W = pv * onehot(dst) (DVE); H += W^T @ [e|1] + W^T @ v (PE).
  - Iter 1 computes e = [msg|cos(time enc)] @ we.T via PE outer-product time
    encoding + Cody-Waite on DVE + Sin on ACT; stores e rows, e^T and the
    dst-onehot to DRAM for iters 2-3.
"""
import sys
import os
import math
import numpy as np

sys.path.insert(0, "/opt/trn_rl_repo")

MEM = 128
NODE = 128
EDGE = 72
TIME = 56
ITERS = 3
EPS = 0.1
GAMMA = 0.1
NCORES = 8
P = 128
GWIN = 2          # windows per edge-phase group
LO_LIMIT = 32768  # int16 dma_gather index limit

INV_SQRT_D = 1.0 / math.sqrt(MEM)
INV_2PI = 1.0 / (2.0 * math.pi)
MAGIC = 12582912.0           # 1.5 * 2**23, round-to-nearest-int trick
C1 = 6.28125                 # 2pi Cody-Waite split, 7 mantissa bits
C2 = 0.0019353071795864769
TWO_PI_F32 = float(np.float32(2.0 * math.pi))
HALF_PI = math.pi / 2.0
PI_CLAMP = float(np.float32(math.pi) * 0.9999995)


def _wrap16(a):
    """int16 index list -> [128, n/16] dma_gather layout."""
    a = np.asarray(a, dtype=np.int16)
    assert len(a) % 16 == 0
    return np.tile(a.reshape(-1, 16).T, (8, 1)).astype(np.int16)


def _host_prep(n_id, edge_index, t, msg, last_update):
    import ml_dtypes
    lu_of_node = np.asarray(last_update)[np.asarray(n_id)]
    N = n_id.shape[0]
    E = edge_index.shape[1]
    src = np.asarray(edge_index[0], dtype=np.int64)
    dst = np.asarray(edge_index[1], dtype=np.int64)

    deg = np.bincount(dst, minlength=N)
    cum = np.cumsum(deg)
    # contiguous node ranges with ~equal edge counts
    bounds = [0]
    for c in range(1, NCORES):
        bounds.append(int(np.searchsorted(cum, E * c / NCORES)))
    bounds.append(N)
    node_core = np.zeros(N, dtype=np.int64)
    for c in range(NCORES):
        node_core[bounds[c]:bounds[c + 1]] = c
    ncnt = [bounds[c + 1] - bounds[c] for c in range(NCORES)]
    NW = max(1, math.ceil(max(ncnt) / P))
    NW = math.ceil(NW / GWIN) * GWIN
    NSH = NW * P
    assert NCORES * NSH - LO_LIMIT < LO_LIMIT, "hi table exceeds int16 range"

    # per-core node order: round-robin by degree into windows
    local_of = np.full(N, -1, dtype=np.int64)
    nid_own = np.zeros((NCORES, NSH), dtype=np.int32)
    for c in range(NCORES):
        nodes = np.arange(bounds[c], bounds[c + 1])
        order = nodes[np.argsort(-deg[nodes], kind="stable")]
        li = np.arange(len(order))
        loc = (li % NW) * P + (li // NW)
        assert loc.max(initial=0) < NSH
        local_of[order] = loc
        nid_own[c, loc] = n_id[order]
    glob_row = node_core * NSH + local_of  # kv_full row of each original node

    # edges per core, windowed, lo/hi split
    e_core = node_core[dst]
    ld_all = local_of[dst]          # 0..NSH-1 within dst core
    e_win = ld_all // P
    srcrow = glob_row[src]
    is_lo = srcrow < LO_LIMIT

    KL = 0
    KH = 0
    per_core_win_edges = []
    for c in range(NCORES):
        m = e_core == c
        wins = []
        for w in range(NW):
            mw = m & (e_win == w)
            elo = np.nonzero(mw & is_lo)[0]
            ehi = np.nonzero(mw & ~is_lo)[0]
            wins.append((elo, ehi))
            KL = max(KL, math.ceil(len(elo) / P))
            KH = max(KH, math.ceil(len(ehi) / P))
        per_core_win_edges.append(wins)
    NCH_W = KL + KH
    EP = NW * NCH_W * P            # padded edges per core
    ELO = NW * KL * P
    EHI = NW * KH * P

    cores = []
    for c in range(NCORES):
        msgT_sh = np.zeros((EDGE, EP), dtype=np.float32)  # split a/b below
        rel_sh = np.zeros((EP, 1), dtype=np.float32)
        ld_sh = np.full((EP, 1), -1.0, dtype=np.float32)  # reshaped below
        kvlo = np.zeros(max(ELO, 16), dtype=np.int16)
        kvhi = np.zeros(max(EHI, 16), dtype=np.int16)
        for w in range(NW):
            elo, ehi = per_core_win_edges[c][w]
            for which, elist, K, base_k, kvarr, kbase in (
                (0, elo, KL, 0, kvlo, w * KL * P),
                (1, ehi, KH, KL, kvhi, w * KH * P),
            ):
                if K == 0:
                    continue
                n = len(elist)
                pos0 = (w * NCH_W + base_k) * P
                pos = pos0 + np.arange(n)
                msgT_sh[:, pos] = msg[elist].T
                rel_sh[pos, 0] = np.abs(
                    lu_of_node[src[elist]] - t[elist]).astype(np.float32)
                ld_sh[pos, 0] = (ld_all[elist] % P).astype(np.float32)
                rows = srcrow[elist] - (LO_LIMIT if which else 0)
                kvarr[kbase:kbase + n] = rows.astype(np.int16)
        mbf = msgT_sh.astype(ml_dtypes.bfloat16)
        cores.append(dict(
            msgTa=np.ascontiguousarray(mbf[0:64]),
            msgTb=np.ascontiguousarray(mbf[64:EDGE]),
            rel=rel_sh,
            ld=np.ascontiguousarray(ld_sh.reshape(-1, P).T),
            kvlo=_wrap16(kvlo), kvhi=_wrap16(kvhi),
            nid=nid_own[c].reshape(NSH, 1),
        ))

    meta = dict(N=N, E=E, NSH=NSH, NW=NW, KL=KL, KH=KH, NCH_W=NCH_W, EP=EP,
                ELO=max(ELO, 16), EHI=max(EHI, 16),
                bounds=bounds, local_of=local_of)
    return cores, meta


def _build(meta, num_nodes):
    import concourse.bacc as bacc
    import concourse.bass as bass
    import concourse.mybir as mybir
    import concourse.tile as tile
    from concourse.masks import make_identity

    dt = mybir.dt
    Alu = mybir.AluOpType
    Act = mybir.ActivationFunctionType

    NSH, NW, KL, KH, NCH_W, EP = (meta[k] for k in
                                  ("NSH", "NW", "KL", "KH", "NCH_W", "EP"))
    ELO, EHI = meta["ELO"], meta["EHI"]
    NFULL = NCORES * NSH
    NGRP = NW // GWIN
    GN = GWIN * NCH_W      # chunks per group

    nc = bacc.Bacc("TRN2", target_bir_lowering=False, debug=False,
                   num_devices=NCORES)

    def din(name, shape, dtype):
        return nc.dram_tensor(name, shape, dtype, kind="ExternalInput")

    t_mem = din("memory", [num_nodes, MEM], dt.float32)
    t_stat = din("static_node_features", [num_nodes, NODE], dt.float32)
    t_lu = din("last_update", [num_nodes, 1], dt.int32)
    t_nid = din("nid", [NSH, 1], dt.int32)
    t_msgTa = din("msgTa", [64, EP], dt.bfloat16)
    t_msgTb = din("msgTb", [EDGE - 64, EP], dt.bfloat16)
    t_rel = din("rel", [EP, 1], dt.float32)
    t_ld = din("ld", [P, EP // P], dt.float32)
    t_kvlo = din("kvlo", [P, ELO // 16], dt.int16)
    t_kvhi = din("kvhi", [P, EHI // 16], dt.int16)
    # host-pretransposed weights
    t_encwT = din("enc_wT", [MEM + NODE, MEM], dt.float32)
    t_wqT = din("wqT", [MEM, MEM], dt.float32)
    t_wkT = din("wkT", [MEM, MEM], dt.float32)
    t_wvT = din("wvT", [MEM, MEM], dt.float32)
    t_weT = din("weT", [EDGE + TIME, MEM], dt.float32)
    t_aw = din("aW", [MEM, MEM], dt.float32)
    t_awT = din("aWT", [MEM, MEM], dt.float32)
    t_brow = din("brow", [1, 4 * MEM], dt.float32)   # [bq|bk|bv|abias]
    t_encb = din("encb", [1, MEM], dt.float32)
    t_bqcol = din("bqcol", [MEM, 1], dt.float32)
    t_twrow = din("twrow", [1, TIME], dt.float32)
    # time-encoding per-partition columns (56 rows)
    t_tbq = din("tbq", [TIME, 1], dt.float32)     # tb/2pi + 0.25
    t_tbhp = din("tbhp", [TIME, 1], dt.float32)   # tb + pi/2
    t_clhi = din("clhi", [TIME, 1], dt.float32)   # PI_CLAMP - tbhp
    t_cllo = din("cllo", [TIME, 1], dt.float32)   # -PI_CLAMP - tbhp
    t_out = nc.dram_tensor("out", [NSH, MEM], dt.float32, kind="ExternalOutput")

    with tile.TileContext(nc) as tc:
        perm = tc.alloc_tile_pool(name="perm", bufs=1)
        sb = tc.alloc_tile_pool(name="sb", bufs=2)        # group tiles
        sb3 = tc.alloc_tile_pool(name="sb3", bufs=3)      # per-chunk tiles
        ps = tc.alloc_tile_pool(name="ps", bufs=2, space="PSUM")    # transposes
        psq = tc.alloc_tile_pool(name="psq", bufs=2, space="PSUM")  # 2KB mm
        psa = tc.alloc_tile_pool(name="psa", bufs=2, space="PSUM")  # AL / eT / er
        dram = tc.alloc_tile_pool(name="dram", bufs=1, space="DRAM")

        # ---------- persistent DRAM ----------
        kv_own = dram.tile([NSH, 2 * MEM], dt.bfloat16)
        kv1_full = dram.tile([NFULL, 2 * MEM], dt.bfloat16, addr_space="Shared")
        kv2_full = dram.tile([NFULL, 2 * MEM], dt.bfloat16, addr_space="Shared")
        kv3_full = dram.tile([NFULL, 2 * MEM], dt.bfloat16, addr_space="Shared")
        e_dram = dram.tile([P, (EP // P) * 132], dt.bfloat16)  # e | 1.0 | pad
        eT_dram = dram.tile([P, EP], dt.bfloat16)        # e^T chunk-blocked
        oh_dram = dram.tile([P, EP], dt.bfloat16)        # onehot(dst)

        # ---------- persistent SBUF ----------
        x_sb = perm.tile([P, NW, MEM], dt.float32)
        xa_sb = perm.tile([P, NW, MEM], dt.float32)
        qT_sb = perm.tile([P, NW, MEM], dt.bfloat16)
        nid_sb = perm.tile([P, NW], dt.int32)
        iota_f = perm.tile([P, P], dt.float32)
        ident = perm.tile([P, P], dt.float32)
        wcat = perm.tile([MEM, 4 * MEM], dt.float32)     # [wqT|wkT|wvT|A_rhs]
        weT_bf = perm.tile([P, MEM], dt.bfloat16)
        encwT_sb = perm.tile([P, 2, MEM], dt.float32)
        bias_full = perm.tile([P, 4 * MEM], dt.float32)
        encb_full = perm.tile([P, MEM], dt.float32)
        tw_row = perm.tile([1, TIME], dt.float32)
        bq_col = perm.tile([P, 1], dt.float32)
        tbq_col = perm.tile([P, 1], dt.float32)
        tbhp_col = perm.tile([P, 1], dt.float32)
        clhi_col = perm.tile([P, 1], dt.float32)
        cllo_col = perm.tile([P, 1], dt.float32)
        zero_col = perm.tile([P, 1], dt.float32)

        TLO = 64         # te partition base (aligned)
        THI = TLO + TIME

        # ---------- startup constants ----------
        make_identity(nc, ident[:])
        ident_bf = perm.tile([P, P], dt.bfloat16)
        nc.vector.tensor_copy(out=ident_bf[:], in_=ident[:])
        ii = perm.tile([P, P], dt.int32)
        nc.gpsimd.iota(ii[:, :], pattern=[[1, P]], base=0, channel_multiplier=0)
        nc.vector.tensor_copy(out=iota_f[:], in_=ii[:, :])
        nc.vector.memset(zero_col[:], 0.0)

        nc.sync.dma_start(out=nid_sb[:], in_=t_nid.ap().rearrange(
            "(c p) one -> p (c one)", p=P))
        nc.sync.dma_start(out=wcat[:, 0:MEM], in_=t_wqT[:])
        nc.sync.dma_start(out=wcat[:, MEM:2 * MEM], in_=t_wkT[:])
        nc.sync.dma_start(out=wcat[:, 2 * MEM:3 * MEM], in_=t_wvT[:])
        nc.sync.dma_start(out=encwT_sb[:, 0, :], in_=t_encwT[0:P, :])
        nc.sync.dma_start(out=encwT_sb[:, 1, :], in_=t_encwT[P:2 * P, :])
        nc.sync.dma_start(out=tw_row[:], in_=t_twrow[:])
        nc.sync.dma_start(out=bq_col[:], in_=t_bqcol[:])
        nc.sync.dma_start(out=tbq_col[TLO:THI, :], in_=t_tbq[:])
        nc.sync.dma_start(out=tbhp_col[TLO:THI, :], in_=t_tbhp[:])
        nc.sync.dma_start(out=clhi_col[TLO:THI, :], in_=t_clhi[:])
        nc.sync.dma_start(out=cllo_col[TLO:THI, :], in_=t_cllo[:])
        # A_rhs[f, j] = aW.T - aW - gamma*I  (in [f, j] layout)
        awt_sb = sb3.tile([P, MEM], dt.float32, tag="awt")
        aw_sb = sb3.tile([P, MEM], dt.float32, tag="aw")
        nc.sync.dma_start(out=awt_sb[:], in_=t_awT[:])
        nc.sync.dma_start(out=aw_sb[:], in_=t_aw[:])
        nc.vector.tensor_tensor(out=awt_sb[:], in0=awt_sb[:], in1=aw_sb[:],
                                op=Alu.subtract)
        gi = sb3.tile([P, MEM], dt.float32, tag="gi")
        nc.vector.tensor_scalar(out=gi[:], in0=ident[:], scalar1=GAMMA,
                                scalar2=None, op0=Alu.mult)
        nc.vector.tensor_tensor(out=wcat[:, 3 * MEM:4 * MEM], in0=awt_sb[:],
                                in1=gi[:], op=Alu.subtract)
        we_sb = sb3.tile([P, MEM], dt.float32, tag="wesb")
        nc.sync.dma_start(out=we_sb[:], in_=t_weT[:])
        nc.vector.tensor_copy(out=weT_bf[:], in_=we_sb[:])

        # partition-replicate small row vectors via ones-outer-product
        ones_row = perm.tile([1, P], dt.float32)
        nc.vector.memset(ones_row[:], 1.0)

        def replicate(dst_ap, src_dram_ap, width):
            row = sb3.tile([1, 4 * MEM], dt.float32, tag="reprow")
            nc.sync.dma_start(out=row[:, :width], in_=src_dram_ap)
            op = psq.tile([P, 4 * MEM], dt.float32, space="PSUM", tag="qp")
            nc.tensor.matmul(out=op[:, :width], lhsT=ones_row[:],
                             rhs=row[:, :width], start=True, stop=True)
            nc.vector.tensor_copy(out=dst_ap, in_=op[:, :width])

        replicate(bias_full[:], t_brow[:], 4 * MEM)
        replicate(encb_full[:], t_encb[:], MEM)

        # ---------- encoder: x = [memory|static][n_id] @ enc_w.T + enc_b ----------
        for c in range(NW):
            memg = sb3.tile([P, MEM], dt.float32, tag="memg")
            statg = sb3.tile([P, NODE], dt.float32, tag="statg")
            nc.gpsimd.indirect_dma_start(
                out=memg[:], out_offset=None, in_=t_mem[:],
                in_offset=bass.IndirectOffsetOnAxis(ap=nid_sb[:, c:c + 1], axis=0))
            nc.gpsimd.indirect_dma_start(
                out=statg[:], out_offset=None, in_=t_stat[:],
                in_offset=bass.IndirectOffsetOnAxis(ap=nid_sb[:, c:c + 1], axis=0))
            xps = psq.tile([P, 4 * MEM], dt.float32, space="PSUM", tag="qp")
            for h, g in enumerate((memg, statg)):
                tp = ps.tile([P, P], dt.float32, space="PSUM", tag="tp")
                nc.tensor.transpose(out=tp[:], in_=g[:], identity=ident[:])
                gt = sb3.tile([P, P], dt.float32, tag="gt")
                nc.vector.tensor_copy(out=gt[:], in_=tp[:])
                nc.tensor.matmul(out=xps[:, 0:MEM], lhsT=gt[:], rhs=encwT_sb[:, h, :],
                                 start=(h == 0), stop=(h == 1))
            nc.vector.tensor_tensor(out=x_sb[:, c, :], in0=xps[:, 0:MEM],
                                    in1=encb_full[:], op=Alu.add)

        # ---------- iterations ----------
        for it in range(ITERS):
            first = it == 0
            kvo = kv_own
            kvf = (kv1_full, kv2_full, kv3_full)[it]
            kvw = 2 * MEM

            # node phase
            for c in range(NW):
                tp = ps.tile([P, P], dt.float32, space="PSUM", tag="tp")
                nc.tensor.transpose(out=tp[:], in_=x_sb[:, c, :], identity=ident[:])
                xt = sb3.tile([P, P], dt.float32, tag="xt")
                nc.vector.tensor_copy(out=xt[:], in_=tp[:])
                qp = psq.tile([P, 4 * MEM], dt.float32, space="PSUM", tag="qp")
                nc.tensor.matmul(out=qp[:], lhsT=xt[:], rhs=wcat[:],
                                 start=True, stop=True)
                # q^T = wq @ x^T (+bq via ACT), bf16, kept in SBUF
                qtp = ps.tile([P, P], dt.float32, space="PSUM", tag="tp")
                nc.tensor.matmul(out=qtp[:, 0:MEM], lhsT=wcat[:, 0:MEM], rhs=xt[:],
                                 start=True, stop=True)
                nc.scalar.activation(out=qT_sb[:, c, :], in_=qtp[:, 0:MEM],
                                     func=Act.Identity, bias=bq_col[:, 0:1])
                kvbf = sb3.tile([P, 2 * MEM], dt.bfloat16, tag="kvbf")
                nc.vector.tensor_tensor(out=kvbf[:], in0=qp[:, MEM:3 * MEM],
                                        in1=bias_full[:, MEM:3 * MEM], op=Alu.add)
                nc.vector.tensor_tensor(out=xa_sb[:, c, :], in0=qp[:, 3 * MEM:],
                                        in1=bias_full[:, 3 * MEM:], op=Alu.add)
                kvo_r = kvo[:].rearrange("(c p) f -> p c f", p=P)
                nc.sync.dma_start(out=kvo_r[:, c, 0:2 * MEM], in_=kvbf[:])

            nc.gpsimd.collective_compute(
                "AllGather", mybir.AluOpType.bypass,
                replica_groups=[list(range(NCORES))],
                ins=[kvo.opt()], outs=[kvf.opt()])

            # edge phase
            for g in range(NGRP):
                c0 = g * GN                    # first global chunk col
                ldt = sb.tile([P, GN], dt.float32, tag="ldt")
                nc.sync.dma_start(out=ldt[:], in_=t_ld[:, c0:c0 + GN])
                # gather kv rows for the group
                kvg = {}
                for which, K, tix, lim0, lim1 in (
                        (0, KL, t_kvlo, 0, min(LO_LIMIT, NFULL)),
                        (1, KH, t_kvhi, LO_LIMIT, NFULL)):
                    if K == 0:
                        continue
                    gk = GWIN * K
                    kk0 = g * gk
                    kix = sb.tile([P, gk * 8], dt.int16, tag=f"kix{which}")
                    nc.sync.dma_start(out=kix[:], in_=tix[:, kk0 * 8:(kk0 + gk) * 8])
                    kt = sb.tile([P, gk, kvw], dt.bfloat16, tag=f"kvg{which}")
                    kmax = 8
                    for b0 in range(0, gk, kmax):
                        b1 = min(b0 + kmax, gk)
                        nc.gpsimd.dma_gather(
                            kt[:, b0:b1, :], kvf[lim0:lim1, :],
                            kix[:, b0 * 8:b1 * 8],
                            (b1 - b0) * P, (b1 - b0) * P, kvw)
                    kvg[which] = kt

                def kv_of(tc_):
                    w, k = tc_ // NCH_W, tc_ % NCH_W
                    if k < KL:
                        return kvg[0], w * KL + k
                    return kvg[1], w * KH + (k - KL)

                if first:
                    # --- pass A: time encoding + e / e^T / onehot ---
                    attrT = sb.tile([P, GN, P], dt.bfloat16, tag="attrT")
                    nc.sync.dma_start(
                        out=attrT[0:64, :, :],
                        in_=t_msgTa[:, c0 * P:(c0 + GN) * P])
                    nc.sync.dma_start(
                        out=attrT[THI:P, :, :],
                        in_=t_msgTb[:, c0 * P:(c0 + GN) * P])
                    et_g = sb.tile([P, GN, 132], dt.bfloat16, tag="et")
                    eT_g = sb.tile([P, GN, P], dt.bfloat16, tag="eT")
                    oh_g = sb.tile([P, GN, P], dt.bfloat16, tag="oh")
                    nc.vector.memset(et_g[:, :, MEM:MEM + 1], 1.0)
                    nc.vector.memset(et_g[:, :, MEM + 1:132], 0.0)

                    for b0 in range(0, GN, 4):
                        b1 = min(b0 + 4, GN)
                        bn = b1 - b0
                        rel_row = sb.tile([1, 4 * P], dt.float32, tag="relrow")
                        nc.sync.dma_start(
                            out=rel_row[:, 0:bn * P],
                            in_=t_rel[(c0 + b0) * P:(c0 + b1) * P, :])
                        u0 = psq.tile([P, 4 * P], dt.float32, space="PSUM",
                                      tag="qp")
                        nc.tensor.matmul(out=u0[TLO:THI, 0:bn * P],
                                         lhsT=tw_row[:],
                                         rhs=rel_row[:, 0:bn * P],
                                         start=True, stop=True)
                        m_sb = sb.tile([P, 4 * P], dt.float32, tag="msb")
                        nc.scalar.activation(
                            out=m_sb[TLO:THI, 0:bn * P],
                            in_=u0[TLO:THI, 0:bn * P], func=Act.Identity,
                            scale=INV_2PI, bias=tbq_col[TLO:THI, 0:1])
                        nc.vector.tensor_scalar(
                            out=m_sb[TLO:THI, 0:bn * P], in0=m_sb[TLO:THI, 0:bn * P],
                            scalar1=MAGIC, scalar2=-MAGIC,
                            op0=Alu.add, op1=Alu.add)
                        v0 = sb.tile([P, 4 * P], dt.float32, tag="v0")
                        nc.vector.scalar_tensor_tensor(
                            out=v0[TLO:THI, 0:bn * P], in0=m_sb[TLO:THI, 0:bn * P],
                            scalar=-TWO_PI_F32, in1=u0[TLO:THI, 0:bn * P],
                            op0=Alu.mult, op1=Alu.add)
                        nc.vector.tensor_scalar(
                            out=v0[TLO:THI, 0:bn * P], in0=v0[TLO:THI, 0:bn * P],
                            scalar1=clhi_col[TLO:THI, 0:1],
                            scalar2=cllo_col[TLO:THI, 0:1],
                            op0=Alu.min, op1=Alu.max)
                        nc.scalar.activation(
                            out=attrT[TLO:THI, b0:b1, :], in_=v0[TLO:THI, 0:bn * P],
                            func=Act.Sin, bias=tbhp_col[TLO:THI, 0:1])
                    for b0 in range(0, GN, 4):
                        b1 = min(b0 + 4, GN)
                        bn = b1 - b0
                        etp = psa.tile([P, 4, P], dt.float32, space="PSUM",
                                       tag="alp", name="etp")
                        for j in range(bn):
                            nc.tensor.matmul(out=etp[:, j, :], lhsT=weT_bf[:],
                                             rhs=attrT[:, b0 + j, :],
                                             start=True, stop=True)
                        nc.scalar.activation(out=eT_g[:, b0:b1, :],
                                             in_=etp[:, 0:bn, :],
                                             func=Act.Identity)
                        erp = psa.tile([P, 4, P], dt.float32, space="PSUM",
                                       tag="alp", name="erp")
                        for j in range(bn):
                            nc.tensor.matmul(out=erp[:, j, :],
                                             lhsT=attrT[:, b0 + j, :],
                                             rhs=weT_bf[:], start=True, stop=True)
                        nc.scalar.activation(out=et_g[:, b0:b1, 0:MEM],
                                             in_=erp[:, 0:bn, :],
                                             func=Act.Identity)
                        for j in range(bn):
                            nc.vector.tensor_tensor(
                                out=oh_g[:, b0 + j, :],
                                in0=ldt[:, b0 + j:b0 + j + 1].to_broadcast([P, P]),
                                in1=iota_f[:], op=Alu.is_equal)
                    # store for iters 2-3
                    nc.sync.dma_start(
                        out=eT_dram[:, c0 * P:(c0 + GN) * P], in_=eT_g[:])
                    nc.sync.dma_start(
                        out=e_dram[:, c0 * 132:(c0 + GN) * 132], in_=et_g[:])
                    nc.sync.dma_start(
                        out=oh_dram[:, c0 * P:(c0 + GN) * P], in_=oh_g[:])
                else:
                    et_g = sb.tile([P, GN, 132], dt.bfloat16, tag="et")
                    nc.sync.dma_start(
                        out=et_g[:], in_=e_dram[:, c0 * 132:(c0 + GN) * 132])
                    eT_g = sb.tile([P, GN, P], dt.bfloat16, tag="eT")
                    nc.sync.dma_start(
                        out=eT_g[:], in_=eT_dram[:, c0 * P:(c0 + GN) * P])
                    oh_g = sb.tile([P, GN, P], dt.bfloat16, tag="oh")
                    nc.sync.dma_start(
                        out=oh_g[:], in_=oh_dram[:, c0 * P:(c0 + GN) * P])

                # --- pass B: alpha + softmax scatter ---
                Hs = []
                for w in range(GWIN):
                    Hw = ps.tile([P, 132], dt.float32, space="PSUM",
                                 tag="tp", name=f"H{w}")
                    Hs.append(Hw)
                for w in range(GWIN):
                    H = Hs[w]
                    for b0 in range(0, NCH_W, 4):
                        b1 = min(b0 + 4, NCH_W)
                        bn = b1 - b0
                        t0 = w * NCH_W + b0
                        ktp = ps.tile([P, 4, P], dt.bfloat16, space="PSUM",
                                      tag="tpbf")
                        for j in range(bn):
                            kt, kc = kv_of(t0 + j)
                            nc.tensor.transpose(out=ktp[:, j, :],
                                                in_=kt[:, kc, 0:MEM],
                                                identity=ident_bf[:])
                        kseT = sb3.tile([P, 4, P], dt.bfloat16, tag="kseT")
                        nc.vector.tensor_tensor(
                            out=kseT[:, 0:bn, :], in0=ktp[:, 0:bn, :],
                            in1=eT_g[:, t0:t0 + bn, :], op=Alu.add)
                        alp = psa.tile([P, 4, P], dt.float32, space="PSUM",
                                       tag="alp")
                        for j in range(bn):
                            nc.tensor.matmul(out=alp[:, j, :],
                                             lhsT=kseT[:, j, :],
                                             rhs=qT_sb[:, g * GWIN + w, :],
                                             start=True, stop=True)
                        pv = sb3.tile([P, 4, P], dt.bfloat16, tag="pv")
                        nc.scalar.activation(out=pv[:, 0:bn, :],
                                             in_=alp[:, 0:bn, :], func=Act.Exp,
                                             scale=INV_SQRT_D)
                        W = sb3.tile([P, 4, P], dt.bfloat16, tag="W")
                        nc.vector.tensor_tensor(out=W[:, 0:bn, :],
                                                in0=pv[:, 0:bn, :],
                                                in1=oh_g[:, t0:t0 + bn, :],
                                                op=Alu.mult)
                        for j in range(bn):
                            kt, kc = kv_of(t0 + j)
                            k = b0 + j
                            nc.tensor.matmul(out=H[:], lhsT=W[:, j, :],
                                             rhs=et_g[:, t0 + j, :],
                                             start=(k == 0), stop=False)
                            nc.tensor.matmul(out=H[:, 0:MEM], lhsT=W[:, j, :],
                                             rhs=kt[:, kc, MEM:2 * MEM],
                                             start=False,
                                             stop=(k == NCH_W - 1))

                # --- window updates ---
                for w in range(GWIN):
                    wg = g * GWIN + w
                    H = Hs[w]
                    sden = sb3.tile([P, 1], dt.float32, tag="sden")
                    nc.vector.tensor_scalar(out=sden[:], in0=H[:, MEM:MEM + 1],
                                            scalar1=1e-30, scalar2=None,
                                            op0=Alu.max)
                    nc.vector.reciprocal(out=sden[:], in_=sden[:])
                    hx = sb3.tile([P, MEM], dt.float32, tag="hx")
                    nc.vector.affine_then_add(out=hx[:], in0=H[:, 0:MEM],
                                              in1=xa_sb[:, wg, :],
                                              scale=sden[:, 0:1], bias=0.0)
                    nc.scalar.activation(out=hx[:], in_=hx[:], func=Act.Tanh)
                    nc.vector.scalar_tensor_tensor(
                        out=x_sb[:, wg, :], in0=hx[:], scalar=EPS,
                        in1=x_sb[:, wg, :], op0=Alu.mult, op1=Alu.add)

        nc.sync.dma_start(
            out=t_out.ap().rearrange("(c p) f -> p c f", p=P),
            in_=x_sb[:])

        for _pool in (dram, psa, psq, ps, sb3, sb, perm):
            _pool.release()

    nc.compile()
    return nc


def kernel(n_id, edge_index, t, msg, static_node_features, memory, last_update,
           enc_w, enc_b, time_w, time_b, wq, bq, wk, bk, wv, bv, we, aW, abias):
    from concourse import bass_utils

    n_id = np.asarray(n_id)
    edge_index = np.asarray(edge_index)
    t = np.asarray(t)
    msg = np.asarray(msg, dtype=np.float32)
    num_nodes = memory.shape[0]

    cores, meta = _host_prep(n_id, edge_index, t, msg, last_update)
    nc = _build(meta, num_nodes)

    time_w = np.asarray(time_w, dtype=np.float32)
    time_b = np.asarray(time_b, dtype=np.float32)
    brow = np.concatenate([np.asarray(bq), np.asarray(bk), np.asarray(bv),
                           np.asarray(abias)]).reshape(1, -1).astype(np.float32)
    shared = {
        "memory": np.asarray(memory, dtype=np.float32),
        "static_node_features": np.asarray(static_node_features, dtype=np.float32),
        "last_update": np.asarray(last_update, dtype=np.int32).reshape(-1, 1),
        "enc_wT": np.ascontiguousarray(np.asarray(enc_w, dtype=np.float32).T),
        "wqT": np.ascontiguousarray(np.asarray(wq, dtype=np.float32).T),
        "wkT": np.ascontiguousarray(np.asarray(wk, dtype=np.float32).T),
        "wvT": np.ascontiguousarray(np.asarray(wv, dtype=np.float32).T),
        "weT": np.ascontiguousarray(np.concatenate([
            np.asarray(we, dtype=np.float32).T[0:64],
            np.asarray(we, dtype=np.float32).T[72:128],
            np.asarray(we, dtype=np.float32).T[64:72]], axis=0)),
        "aW": np.asarray(aW, dtype=np.float32),
        "aWT": np.ascontiguousarray(np.asarray(aW, dtype=np.float32).T),
        "brow": brow,
        "encb": np.asarray(enc_b, dtype=np.float32).reshape(1, -1),
        "bqcol": np.asarray(bq, dtype=np.float32).reshape(-1, 1),
        "twrow": time_w.reshape(1, -1),
        "tbq": (time_b * np.float32(INV_2PI) + np.float32(0.25)
                ).astype(np.float32).reshape(-1, 1),
        "tbhp": (time_b + np.float32(HALF_PI)).astype(np.float32).reshape(-1, 1),
        "clhi": (np.float32(PI_CLAMP) - time_b - np.float32(HALF_PI)
                 ).astype(np.float32).reshape(-1, 1),
        "cllo": (-np.float32(PI_CLAMP) - time_b - np.float32(HALF_PI)
                 ).astype(np.float32).reshape(-1, 1),
    }
    in_maps = []
    for c in range(NCORES):
        m = dict(shared)
        m["nid"] = cores[c]["nid"]
        m["msgTa"] = cores[c]["msgTa"]
        m["msgTb"] = cores[c]["msgTb"]
        m["rel"] = cores[c]["rel"]
        m["ld"] = cores[c]["ld"]
        m["kvlo"] = cores[c]["kvlo"]
        m["kvhi"] = cores[c]["kvhi"]
        in_maps.append(m)

    if os.environ.get("KERNEL_SIM", "0") == "1":
        from concourse.bass_interp import MultiCoreSim
        sim = MultiCoreSim(nc, num_cores=NCORES, trace=False,
                           require_finite=False, require_nnan=False)
        cs = list(sim.cores.values())
        for ci, core in enumerate(cs):
            for k, v in in_maps[ci].items():
                core.tensor(k)[:] = v
        sim.simulate(check_with_hw=False, trace_hw=False)

        class R:
            results = [{"out": np.asarray(core.tensor("out"))} for core in cs]
        res = R()
        kernel.last_exec_time_ns = None
        N = meta["N"]
        local_of = meta["local_of"]
        bounds = meta["bounds"]
        out = np.zeros((N, MEM), dtype=np.float32)
        for c in range(NCORES):
            nodes = np.arange(bounds[c], bounds[c + 1])
            out[nodes] = res.results[c]["out"][local_of[nodes]]
        return out

    trace = os.environ.get("KERNEL_TRACE", "0") == "1"
    res = bass_utils.run_bass_kernel_spmd(
        nc, in_maps, core_ids=list(range(NCORES)), trace=trace)
    if trace:
        print("HW exec time:", res.exec_time_ns, "ns")
        kernel.last_exec_time_ns = res.exec_time_ns
        kernel.last_trace = res.instructions_and_trace

    # unshard: core c's rows [local] -> original node id order
    N = meta["N"]
    local_of = meta["local_of"]
    bounds = meta["bounds"]
    out = np.zeros((N, MEM), dtype=np.float32)
    for c in range(NCORES):
        nodes = np.arange(bounds[c], bounds[c + 1])
        out[nodes] = res.results[c]["out"][local_of[nodes]]
    return out


# revision 20
# speedup vs baseline: 1.0467x; 1.0467x over previous
"""CTAN (gnn_message_passing) Trainium2 kernel — 8 NeuronCores, edge-parallel.

v2: alpha/softmax moved off DVE onto PE/ACT.
  - Host: shard nodes into 8 contiguous ranges balanced by in-degree; edges go
    to the core owning their dst, windowed by dst into 128-node windows,
    lo/hi src-row split for int16 dma_gather, padded to 128-edge chunks.
  - Node phase: per window, x -> q^T (kept in SBUF, bf16), k|v (bf16, DRAM,
    AllGather), xa = x@A.T.
  - Edge phase per chunk: dma_gather kv[src] rows; PE-transpose k; kse^T =
    k^T + e^T (DVE); AL[e,n] = kse^T^T @ q^T_win (PE); pv = exp(scale*AL)
    (ACT); W = pv * onehot(dst) (DVE); H += W^T @ [e|1] + W^T @ v (PE).
  - Iter 1 computes e = [msg|cos(time enc)] @ we.T via PE outer-product time
    encoding + Cody-Waite on DVE + Sin on ACT; stores e rows, e^T and the
    dst-onehot to DRAM for iters 2-3.
"""
import sys
import os
import math
import numpy as np

sys.path.insert(0, "/opt/trn_rl_repo")

MEM = 128
NODE = 128
EDGE = 72
TIME = 56
ITERS = 3
EPS = 0.1
GAMMA = 0.1
NCORES = 8
P = 128
GWIN = 2          # windows per edge-phase group
LO_LIMIT = 32768  # int16 dma_gather index limit

INV_SQRT_D = 1.0 / math.sqrt(MEM)
INV_2PI = 1.0 / (2.0 * math.pi)
MAGIC = 12582912.0           # 1.5 * 2**23, round-to-nearest-int trick
C1 = 6.28125                 # 2pi Cody-Waite split, 7 mantissa bits
C2 = 0.0019353071795864769
TWO_PI_F32 = float(np.float32(2.0 * math.pi))
HALF_PI = math.pi / 2.0
PI_CLAMP = float(np.float32(math.pi) * 0.9999995)


def _wrap16(a):
    """int16 index list -> [128, n/16] dma_gather layout."""
    a = np.asarray(a, dtype=np.int16)
    assert len(a) % 16 == 0
    return np.tile(a.reshape(-1, 16).T, (8, 1)).astype(np.int16)


def _host_prep(n_id, edge_index, t, msg, last_update):
    import ml_dtypes
    lu_of_node = np.asarray(last_update)[np.asarray(n_id)]
    N = n_id.shape[0]
    E = edge_index.shape[1]
    src = np.asarray(edge_index[0], dtype=np.int64)
    dst = np.asarray(edge_index[1], dtype=np.int64)

    deg = np.bincount(dst, minlength=N)
    cum = np.cumsum(deg)
    # contiguous node ranges with ~equal edge counts
    bounds = [0]
    for c in range(1, NCORES):
        bounds.append(int(np.searchsorted(cum, E * c / NCORES)))
    bounds.append(N)
    node_core = np.zeros(N, dtype=np.int64)
    for c in range(NCORES):
        node_core[bounds[c]:bounds[c + 1]] = c
    ncnt = [bounds[c + 1] - bounds[c] for c in range(NCORES)]
    NW = max(1, math.ceil(max(ncnt) / P))
    NW = math.ceil(NW / GWIN) * GWIN
    NSH = NW * P
    assert NCORES * NSH - LO_LIMIT < LO_LIMIT, "hi table exceeds int16 range"

    # per-core node order: round-robin by degree into windows
    local_of = np.full(N, -1, dtype=np.int64)
    nid_own = np.zeros((NCORES, NSH), dtype=np.int32)
    for c in range(NCORES):
        nodes = np.arange(bounds[c], bounds[c + 1])
        order = nodes[np.argsort(-deg[nodes], kind="stable")]
        li = np.arange(len(order))
        loc = (li % NW) * P + (li // NW)
        assert loc.max(initial=0) < NSH
        local_of[order] = loc
        nid_own[c, loc] = n_id[order]
    glob_row = node_core * NSH + local_of  # kv_full row of each original node

    # edges per core, windowed, lo/hi split
    e_core = node_core[dst]
    ld_all = local_of[dst]          # 0..NSH-1 within dst core
    e_win = ld_all // P
    srcrow = glob_row[src]
    is_lo = srcrow < LO_LIMIT

    KL = 0
    KH = 0
    per_core_win_edges = []
    for c in range(NCORES):
        m = e_core == c
        wins = []
        for w in range(NW):
            mw = m & (e_win == w)
            elo = np.nonzero(mw & is_lo)[0]
            ehi = np.nonzero(mw & ~is_lo)[0]
            wins.append((elo, ehi))
            KL = max(KL, math.ceil(len(elo) / P))
            KH = max(KH, math.ceil(len(ehi) / P))
        per_core_win_edges.append(wins)
    NCH_W = KL + KH
    EP = NW * NCH_W * P            # padded edges per core
    ELO = NW * KL * P
    EHI = NW * KH * P

    cores = []
    for c in range(NCORES):
        msgT_sh = np.zeros((EDGE, EP), dtype=np.float32)  # split a/b below
        rel_sh = np.zeros((EP, 1), dtype=np.float32)
        ld_sh = np.full((EP, 1), -1.0, dtype=np.float32)  # reshaped below
        kvlo = np.zeros(max(ELO, 16), dtype=np.int16)
        kvhi = np.zeros(max(EHI, 16), dtype=np.int16)
        for w in range(NW):
            elo, ehi = per_core_win_edges[c][w]
            for which, elist, K, base_k, kvarr, kbase in (
                (0, elo, KL, 0, kvlo, w * KL * P),
                (1, ehi, KH, KL, kvhi, w * KH * P),
            ):
                if K == 0:
                    continue
                n = len(elist)
                pos0 = (w * NCH_W + base_k) * P
                pos = pos0 + np.arange(n)
                msgT_sh[:, pos] = msg[elist].T
                rel_sh[pos, 0] = np.abs(
                    lu_of_node[src[elist]] - t[elist]).astype(np.float32)
                ld_sh[pos, 0] = (ld_all[elist] % P).astype(np.float32)
                rows = srcrow[elist] - (LO_LIMIT if which else 0)
                kvarr[kbase:kbase + n] = rows.astype(np.int16)
        mbf = msgT_sh.astype(ml_dtypes.bfloat16)
        cores.append(dict(
            msgTa=np.ascontiguousarray(mbf[0:64]),
            msgTb=np.ascontiguousarray(mbf[64:EDGE]),
            rel=rel_sh,
            ld=np.ascontiguousarray(ld_sh.reshape(-1, P).T),
            kvlo=_wrap16(kvlo), kvhi=_wrap16(kvhi),
            nid=nid_own[c].reshape(NSH, 1),
        ))

    meta = dict(N=N, E=E, NSH=NSH, NW=NW, KL=KL, KH=KH, NCH_W=NCH_W, EP=EP,
                ELO=max(ELO, 16), EHI=max(EHI, 16),
                bounds=bounds, local_of=local_of)
    return cores, meta


def _build(meta, num_nodes):
    import concourse.bacc as bacc
    import concourse.bass as bass
    import concourse.mybir as mybir
    import concourse.tile as tile
    from concourse.masks import make_identity

    dt = mybir.dt
    Alu = mybir.AluOpType
    Act = mybir.ActivationFunctionType

    NSH, NW, KL, KH, NCH_W, EP = (meta[k] for k in
                                  ("NSH", "NW", "KL", "KH", "NCH_W", "EP"))
    ELO, EHI = meta["ELO"], meta["EHI"]
    NFULL = NCORES * NSH
    NGRP = NW // GWIN
    GN = GWIN * NCH_W      # chunks per group

    nc = bacc.Bacc("TRN2", target_bir_lowering=False, debug=False,
                   num_devices=NCORES)

    def din(name, shape, dtype):
        return nc.dram_tensor(name, shape, dtype, kind="ExternalInput")

    t_mem = din("memory", [num_nodes, MEM], dt.float32)
    t_stat = din("static_node_features", [num_nodes, NODE], dt.float32)
    t_lu = din("last_update", [num_nodes, 1], dt.int32)
    t_nid = din("nid", [NSH, 1], dt.int32)
    t_msgTa = din("msgTa", [64, EP], dt.bfloat16)
    t_msgTb = din("msgTb", [EDGE - 64, EP], dt.bfloat16)
    t_rel = din("rel", [EP, 1], dt.float32)
    t_ld = din("ld", [P, EP // P], dt.float32)
    t_kvlo = din("kvlo", [P, ELO // 16], dt.int16)
    t_kvhi = din("kvhi", [P, EHI // 16], dt.int16)
    # host-pretransposed weights
    t_encwT = din("enc_wT", [MEM + NODE, MEM], dt.float32)
    t_wqT = din("wqT", [MEM, MEM], dt.float32)
    t_wkT = din("wkT", [MEM, MEM], dt.float32)
    t_wvT = din("wvT", [MEM, MEM], dt.float32)
    t_weT = din("weT", [EDGE + TIME, MEM], dt.float32)
    t_aw = din("aW", [MEM, MEM], dt.float32)
    t_awT = din("aWT", [MEM, MEM], dt.float32)
    t_brow = din("brow", [1, 4 * MEM], dt.float32)   # [bq|bk|bv|abias]
    t_encb = din("encb", [1, MEM], dt.float32)
    t_bqcol = din("bqcol", [MEM, 1], dt.float32)
    t_twrow = din("twrow", [1, TIME], dt.float32)
    # time-encoding per-partition columns (56 rows)
    t_tbq = din("tbq", [TIME, 1], dt.float32)     # tb/2pi + 0.25
    t_tbhp = din("tbhp", [TIME, 1], dt.float32)   # tb + pi/2
    t_clhi = din("clhi", [TIME, 1], dt.float32)   # PI_CLAMP - tbhp
    t_cllo = din("cllo", [TIME, 1], dt.float32)   # -PI_CLAMP - tbhp
    t_out = nc.dram_tensor("out", [NSH, MEM], dt.float32, kind="ExternalOutput")

    with tile.TileContext(nc) as tc:
        perm = tc.alloc_tile_pool(name="perm", bufs=1)
        sb = tc.alloc_tile_pool(name="sb", bufs=2)        # group tiles
        sb3 = tc.alloc_tile_pool(name="sb3", bufs=3)      # per-chunk tiles
        ps = tc.alloc_tile_pool(name="ps", bufs=2, space="PSUM")    # transposes
        psq = tc.alloc_tile_pool(name="psq", bufs=2, space="PSUM")  # 2KB mm
        psa = tc.alloc_tile_pool(name="psa", bufs=2, space="PSUM")  # AL / eT / er
        dram = tc.alloc_tile_pool(name="dram", bufs=1, space="DRAM")

        # ---------- persistent DRAM ----------
        kv_own = dram.tile([NSH, 2 * MEM], dt.bfloat16)
        kv1_full = dram.tile([NFULL, 2 * MEM], dt.bfloat16, addr_space="Shared")
        kv2_full = dram.tile([NFULL, 2 * MEM], dt.bfloat16, addr_space="Shared")
        kv3_full = dram.tile([NFULL, 2 * MEM], dt.bfloat16, addr_space="Shared")
        e_dram = dram.tile([P, (EP // P) * 132], dt.bfloat16)  # e | 1.0 | pad
        eT_dram = dram.tile([P, EP], dt.bfloat16)        # e^T chunk-blocked
        oh_dram = dram.tile([P, EP], dt.bfloat16)        # onehot(dst)

        # ---------- persistent SBUF ----------
        x_sb = perm.tile([P, NW, MEM], dt.float32)
        xa_sb = perm.tile([P, NW, MEM], dt.float32)
        qT_sb = perm.tile([P, NW, MEM], dt.bfloat16)
        nid_sb = perm.tile([P, NW], dt.int32)
        iota_f = perm.tile([P, P], dt.float32)
        ident = perm.tile([P, P], dt.float32)
        wcat = perm.tile([MEM, 4 * MEM], dt.float32)     # [wqT|wkT|wvT|A_rhs]
        weT_bf = perm.tile([P, MEM], dt.bfloat16)
        encwT_sb = perm.tile([P, 2, MEM], dt.float32)
        bias_full = perm.tile([P, 4 * MEM], dt.float32)
        encb_full = perm.tile([P, MEM], dt.float32)
        tw_row = perm.tile([1, TIME], dt.float32)
        bq_col = perm.tile([P, 1], dt.float32)
        tbq_col = perm.tile([P, 1], dt.float32)
        tbhp_col = perm.tile([P, 1], dt.float32)
        clhi_col = perm.tile([P, 1], dt.float32)
        cllo_col = perm.tile([P, 1], dt.float32)
        zero_col = perm.tile([P, 1], dt.float32)

        TLO = 64         # te partition base (aligned)
        THI = TLO + TIME

        # ---------- startup constants ----------
        make_identity(nc, ident[:])
        ident_bf = perm.tile([P, P], dt.bfloat16)
        nc.vector.tensor_copy(out=ident_bf[:], in_=ident[:])
        ii = perm.tile([P, P], dt.int32)
        nc.gpsimd.iota(ii[:, :], pattern=[[1, P]], base=0, channel_multiplier=0)
        nc.vector.tensor_copy(out=iota_f[:], in_=ii[:, :])
        nc.vector.memset(zero_col[:], 0.0)

        nc.sync.dma_start(out=nid_sb[:], in_=t_nid.ap().rearrange(
            "(c p) one -> p (c one)", p=P))
        nc.sync.dma_start(out=wcat[:, 0:MEM], in_=t_wqT[:])
        nc.sync.dma_start(out=wcat[:, MEM:2 * MEM], in_=t_wkT[:])
        nc.sync.dma_start(out=wcat[:, 2 * MEM:3 * MEM], in_=t_wvT[:])
        nc.sync.dma_start(out=encwT_sb[:, 0, :], in_=t_encwT[0:P, :])
        nc.sync.dma_start(out=encwT_sb[:, 1, :], in_=t_encwT[P:2 * P, :])
        nc.sync.dma_start(out=tw_row[:], in_=t_twrow[:])
        nc.sync.dma_start(out=bq_col[:], in_=t_bqcol[:])
        nc.sync.dma_start(out=tbq_col[TLO:THI, :], in_=t_tbq[:])
        nc.sync.dma_start(out=tbhp_col[TLO:THI, :], in_=t_tbhp[:])
        nc.sync.dma_start(out=clhi_col[TLO:THI, :], in_=t_clhi[:])
        nc.sync.dma_start(out=cllo_col[TLO:THI, :], in_=t_cllo[:])
        # A_rhs[f, j] = aW.T - aW - gamma*I  (in [f, j] layout)
        awt_sb = sb3.tile([P, MEM], dt.float32, tag="awt")
        aw_sb = sb3.tile([P, MEM], dt.float32, tag="aw")
        nc.sync.dma_start(out=awt_sb[:], in_=t_awT[:])
        nc.sync.dma_start(out=aw_sb[:], in_=t_aw[:])
        nc.vector.tensor_tensor(out=awt_sb[:], in0=awt_sb[:], in1=aw_sb[:],
                                op=Alu.subtract)
        gi = sb3.tile([P, MEM], dt.float32, tag="gi")
        nc.vector.tensor_scalar(out=gi[:], in0=ident[:], scalar1=GAMMA,
                                scalar2=None, op0=Alu.mult)
        nc.vector.tensor_tensor(out=wcat[:, 3 * MEM:4 * MEM], in0=awt_sb[:],
                                in1=gi[:], op=Alu.subtract)
        we_sb = sb3.tile([P, MEM], dt.float32, tag="wesb")
        nc.sync.dma_start(out=we_sb[:], in_=t_weT[:])
        nc.vector.tensor_copy(out=weT_bf[:], in_=we_sb[:])

        # partition-replicate small row vectors via ones-outer-product
        ones_row = perm.tile([1, P], dt.float32)
        nc.vector.memset(ones_row[:], 1.0)

        def replicate(dst_ap, src_dram_ap, width):
            row = sb3.tile([1, 4 * MEM], dt.float32, tag="reprow")
            nc.sync.dma_start(out=row[:, :width], in_=src_dram_ap)
            op = psq.tile([P, 4 * MEM], dt.float32, space="PSUM", tag="qp")
            nc.tensor.matmul(out=op[:, :width], lhsT=ones_row[:],
                             rhs=row[:, :width], start=True, stop=True)
            nc.vector.tensor_copy(out=dst_ap, in_=op[:, :width])

        replicate(bias_full[:], t_brow[:], 4 * MEM)
        replicate(encb_full[:], t_encb[:], MEM)

        # ---------- encoder: x = [memory|static][n_id] @ enc_w.T + enc_b ----------
        for c in range(NW):
            memg = sb3.tile([P, MEM], dt.float32, tag="memg")
            statg = sb3.tile([P, NODE], dt.float32, tag="statg")
            nc.gpsimd.indirect_dma_start(
                out=memg[:], out_offset=None, in_=t_mem[:],
                in_offset=bass.IndirectOffsetOnAxis(ap=nid_sb[:, c:c + 1], axis=0))
            nc.gpsimd.indirect_dma_start(
                out=statg[:], out_offset=None, in_=t_stat[:],
                in_offset=bass.IndirectOffsetOnAxis(ap=nid_sb[:, c:c + 1], axis=0))
            xps = psq.tile([P, 4 * MEM], dt.float32, space="PSUM", tag="qp")
            for h, g in enumerate((memg, statg)):
                tp = ps.tile([P, P], dt.float32, space="PSUM", tag="tp")
                nc.tensor.transpose(out=tp[:], in_=g[:], identity=ident[:])
                gt = sb3.tile([P, P], dt.float32, tag="gt")
                nc.vector.tensor_copy(out=gt[:], in_=tp[:])
                nc.tensor.matmul(out=xps[:, 0:MEM], lhsT=gt[:], rhs=encwT_sb[:, h, :],
                                 start=(h == 0), stop=(h == 1))
            nc.vector.tensor_tensor(out=x_sb[:, c, :], in0=xps[:, 0:MEM],
                                    in1=encb_full[:], op=Alu.add)

        # ---------- iterations ----------
        for it in range(ITERS):
            first = it == 0
            kvo = kv_own
            kvf = (kv1_full, kv2_full, kv3_full)[it]
            kvw = 2 * MEM

            # node phase
            for c in range(NW):
                tp = ps.tile([P, P], dt.float32, space="PSUM", tag="tp")
                nc.tensor.transpose(out=tp[:], in_=x_sb[:, c, :], identity=ident[:])
                xt = sb3.tile([P, P], dt.float32, tag="xt")
                nc.vector.tensor_copy(out=xt[:], in_=tp[:])
                qp = psq.tile([P, 4 * MEM], dt.float32, space="PSUM", tag="qp")
                nc.tensor.matmul(out=qp[:], lhsT=xt[:], rhs=wcat[:],
                                 start=True, stop=True)
                # q^T = wq @ x^T (+bq via ACT), bf16, kept in SBUF
                qtp = ps.tile([P, P], dt.float32, space="PSUM", tag="tp")
                nc.tensor.matmul(out=qtp[:, 0:MEM], lhsT=wcat[:, 0:MEM], rhs=xt[:],
                                 start=True, stop=True)
                nc.scalar.activation(out=qT_sb[:, c, :], in_=qtp[:, 0:MEM],
                                     func=Act.Identity, bias=bq_col[:, 0:1])
                kvbf = sb3.tile([P, 2 * MEM], dt.bfloat16, tag="kvbf")
                nc.vector.tensor_tensor(out=kvbf[:], in0=qp[:, MEM:3 * MEM],
                                        in1=bias_full[:, MEM:3 * MEM], op=Alu.add)
                nc.vector.tensor_tensor(out=xa_sb[:, c, :], in0=qp[:, 3 * MEM:],
                                        in1=bias_full[:, 3 * MEM:], op=Alu.add)
                kvo_r = kvo[:].rearrange("(c p) f -> p c f", p=P)
                nc.sync.dma_start(out=kvo_r[:, c, 0:2 * MEM], in_=kvbf[:])

            nc.gpsimd.collective_compute(
                "AllGather", mybir.AluOpType.bypass,
                replica_groups=[list(range(NCORES))],
                ins=[kvo.opt()], outs=[kvf.opt()])

            # edge phase
            for g in range(NGRP):
                c0 = g * GN                    # first global chunk col
                ldt = sb.tile([P, GN], dt.float32, tag="ldt")
                nc.sync.dma_start(out=ldt[:], in_=t_ld[:, c0:c0 + GN])
                # gather kv rows for the group
                kvg = {}
                for which, K, tix, lim0, lim1 in (
                        (0, KL, t_kvlo, 0, min(LO_LIMIT, NFULL)),
                        (1, KH, t_kvhi, LO_LIMIT, NFULL)):
                    if K == 0:
                        continue
                    gk = GWIN * K
                    kk0 = g * gk
                    kix = sb.tile([P, gk * 8], dt.int16, tag=f"kix{which}")
                    nc.sync.dma_start(out=kix[:], in_=tix[:, kk0 * 8:(kk0 + gk) * 8])
                    kt = sb.tile([P, gk, kvw], dt.bfloat16, tag=f"kvg{which}")
                    kmax = 8
                    for b0 in range(0, gk, kmax):
                        b1 = min(b0 + kmax, gk)
                        nc.gpsimd.dma_gather(
                            kt[:, b0:b1, :], kvf[lim0:lim1, :],
                            kix[:, b0 * 8:b1 * 8],
                            (b1 - b0) * P, (b1 - b0) * P, kvw)
                    kvg[which] = kt

                def kv_of(tc_):
                    w, k = tc_ // NCH_W, tc_ % NCH_W
                    if k < KL:
                        return kvg[0], w * KL + k
                    return kvg[1], w * KH + (k - KL)

                if first:
                    # --- pass A: time encoding + e / e^T / onehot ---
                    attrT = sb.tile([P, GN, P], dt.bfloat16, tag="attrT")
                    nc.sync.dma_start(
                        out=attrT[0:64, :, :],
                        in_=t_msgTa[:, c0 * P:(c0 + GN) * P])
                    nc.sync.dma_start(
                        out=attrT[THI:P, :, :],
                        in_=t_msgTb[:, c0 * P:(c0 + GN) * P])
                    et_g = sb.tile([P, GN, 132], dt.bfloat16, tag="et")
                    eT_g = sb.tile([P, GN, P], dt.bfloat16, tag="eT")
                    oh_g = sb.tile([P, GN, P], dt.bfloat16, tag="oh")
                    nc.vector.memset(et_g[:, :, MEM:MEM + 1], 1.0)
                    nc.vector.memset(et_g[:, :, MEM + 1:132], 0.0)

                    for b0 in range(0, GN, 4):
                        b1 = min(b0 + 4, GN)
                        bn = b1 - b0
                        rel_row = sb.tile([1, 4 * P], dt.float32, tag="relrow")
                        nc.sync.dma_start(
                            out=rel_row[:, 0:bn * P],
                            in_=t_rel[(c0 + b0) * P:(c0 + b1) * P, :])
                        u0 = psq.tile([P, 4 * P], dt.float32, space="PSUM",
                                      tag="qp")
                        nc.tensor.matmul(out=u0[TLO:THI, 0:bn * P],
                                         lhsT=tw_row[:],
                                         rhs=rel_row[:, 0:bn * P],
                                         start=True, stop=True)
                        m_sb = sb.tile([P, 4 * P], dt.float32, tag="msb")
                        nc.vector.tensor_scalar(
                            out=m_sb[TLO:THI, 0:bn * P],
                            in0=u0[TLO:THI, 0:bn * P], scalar1=INV_2PI,
                            scalar2=tbq_col[TLO:THI, 0:1],
                            op0=Alu.mult, op1=Alu.add)
                        nc.vector.tensor_scalar(
                            out=m_sb[TLO:THI, 0:bn * P], in0=m_sb[TLO:THI, 0:bn * P],
                            scalar1=MAGIC, scalar2=-MAGIC,
                            op0=Alu.add, op1=Alu.add)
                        v0 = sb.tile([P, 4 * P], dt.float32, tag="v0")
                        nc.vector.scalar_tensor_tensor(
                            out=v0[TLO:THI, 0:bn * P], in0=m_sb[TLO:THI, 0:bn * P],
                            scalar=-TWO_PI_F32, in1=u0[TLO:THI, 0:bn * P],
                            op0=Alu.mult, op1=Alu.add)
                        nc.vector.tensor_scalar(
                            out=v0[TLO:THI, 0:bn * P], in0=v0[TLO:THI, 0:bn * P],
                            scalar1=clhi_col[TLO:THI, 0:1],
                            scalar2=cllo_col[TLO:THI, 0:1],
                            op0=Alu.min, op1=Alu.max)
                        nc.scalar.activation(
                            out=attrT[TLO:THI, b0:b1, :], in_=v0[TLO:THI, 0:bn * P],
                            func=Act.Sin, bias=tbhp_col[TLO:THI, 0:1])
                    for b0 in range(0, GN, 4):
                        b1 = min(b0 + 4, GN)
                        bn = b1 - b0
                        etp = psa.tile([P, 4, P], dt.float32, space="PSUM",
                                       tag="alp", name="etp")
                        for j in range(bn):
                            nc.tensor.matmul(out=etp[:, j, :], lhsT=weT_bf[:],
                                             rhs=attrT[:, b0 + j, :],
                                             start=True, stop=True)
                        nc.scalar.activation(out=eT_g[:, b0:b1, :],
                                             in_=etp[:, 0:bn, :],
                                             func=Act.Identity)
                        erp = psa.tile([P, 4, P], dt.float32, space="PSUM",
                                       tag="alp", name="erp")
                        for j in range(bn):
                            nc.tensor.matmul(out=erp[:, j, :],
                                             lhsT=attrT[:, b0 + j, :],
                                             rhs=weT_bf[:], start=True, stop=True)
                        nc.scalar.activation(out=et_g[:, b0:b1, 0:MEM],
                                             in_=erp[:, 0:bn, :],
                                             func=Act.Identity)
                        for j in range(bn):
                            nc.vector.tensor_tensor(
                                out=oh_g[:, b0 + j, :],
                                in0=ldt[:, b0 + j:b0 + j + 1].to_broadcast([P, P]),
                                in1=iota_f[:], op=Alu.is_equal)
                    # store for iters 2-3
                    nc.sync.dma_start(
                        out=eT_dram[:, c0 * P:(c0 + GN) * P], in_=eT_g[:])
                    nc.sync.dma_start(
                        out=e_dram[:, c0 * 132:(c0 + GN) * 132], in_=et_g[:])
                    nc.sync.dma_start(
                        out=oh_dram[:, c0 * P:(c0 + GN) * P], in_=oh_g[:])
                else:
                    et_g = sb.tile([P, GN, 132], dt.bfloat16, tag="et")
                    nc.sync.dma_start(
                        out=et_g[:], in_=e_dram[:, c0 * 132:(c0 + GN) * 132])
                    eT_g = sb.tile([P, GN, P], dt.bfloat16, tag="eT")
                    nc.sync.dma_start(
                        out=eT_g[:], in_=eT_dram[:, c0 * P:(c0 + GN) * P])
                    oh_g = sb.tile([P, GN, P], dt.bfloat16, tag="oh")
                    nc.sync.dma_start(
                        out=oh_g[:], in_=oh_dram[:, c0 * P:(c0 + GN) * P])

                # --- pass B: alpha + softmax scatter ---
                Hs = []
                for w in range(GWIN):
                    Hw = ps.tile([P, 132], dt.float32, space="PSUM",
                                 tag="tp", name=f"H{w}")
                    Hs.append(Hw)
                for w in range(GWIN):
                    H = Hs[w]
                    for b0 in range(0, NCH_W, 4):
                        b1 = min(b0 + 4, NCH_W)
                        bn = b1 - b0
                        t0 = w * NCH_W + b0
                        ktp = ps.tile([P, 4, P], dt.bfloat16, space="PSUM",
                                      tag="tpbf")
                        for j in range(bn):
                            kt, kc = kv_of(t0 + j)
                            nc.tensor.transpose(out=ktp[:, j, :],
                                                in_=kt[:, kc, 0:MEM],
                                                identity=ident_bf[:])
                        kseT = sb3.tile([P, 4, P], dt.bfloat16, tag="kseT")
                        nc.vector.tensor_tensor(
                            out=kseT[:, 0:bn, :], in0=ktp[:, 0:bn, :],
                            in1=eT_g[:, t0:t0 + bn, :], op=Alu.add)
                        alp = psa.tile([P, 4, P], dt.float32, space="PSUM",
                                       tag="alp")
                        for j in range(bn):
                            nc.tensor.matmul(out=alp[:, j, :],
                                             lhsT=kseT[:, j, :],
                                             rhs=qT_sb[:, g * GWIN + w, :],
                                             start=True, stop=True)
                        pv = sb3.tile([P, 4, P], dt.bfloat16, tag="pv")
                        nc.scalar.activation(out=pv[:, 0:bn, :],
                                             in_=alp[:, 0:bn, :], func=Act.Exp,
                                             scale=INV_SQRT_D)
                        W = sb3.tile([P, 4, P], dt.bfloat16, tag="W")
                        nc.vector.tensor_tensor(out=W[:, 0:bn, :],
                                                in0=pv[:, 0:bn, :],
                                                in1=oh_g[:, t0:t0 + bn, :],
                                                op=Alu.mult)
                        for j in range(bn):
                            kt, kc = kv_of(t0 + j)
                            k = b0 + j
                            nc.tensor.matmul(out=H[:], lhsT=W[:, j, :],
                                             rhs=et_g[:, t0 + j, :],
                                             start=(k == 0), stop=False)
                            nc.tensor.matmul(out=H[:, 0:MEM], lhsT=W[:, j, :],
                                             rhs=kt[:, kc, MEM:2 * MEM],
                                             start=False,
                                             stop=(k == NCH_W - 1))

                # --- window updates ---
                for w in range(GWIN):
                    wg = g * GWIN + w
                    H = Hs[w]
                    sden = sb3.tile([P, 1], dt.float32, tag="sden")
                    nc.vector.tensor_scalar(out=sden[:], in0=H[:, MEM:MEM + 1],
                                            scalar1=1e-30, scalar2=None,
                                            op0=Alu.max)
                    nc.vector.reciprocal(out=sden[:], in_=sden[:])
                    hx = sb3.tile([P, MEM], dt.float32, tag="hx")
                    nc.vector.affine_then_add(out=hx[:], in0=H[:, 0:MEM],
                                              in1=xa_sb[:, wg, :],
                                              scale=sden[:, 0:1], bias=0.0)
                    nc.scalar.activation(out=hx[:], in_=hx[:], func=Act.Tanh)
                    nc.vector.scalar_tensor_tensor(
                        out=x_sb[:, wg, :], in0=hx[:], scalar=EPS,
                        in1=x_sb[:, wg, :], op0=Alu.mult, op1=Alu.add)

        nc.sync.dma_start(
            out=t_out.ap().rearrange("(c p) f -> p c f", p=P),
            in_=x_sb[:])

        for _pool in (dram, psa, psq, ps, sb3, sb, perm):
            _pool.release()

    nc.compile()
    return nc


def kernel(n_id, edge_index, t, msg, static_node_features, memory, last_update,
           enc_w, enc_b, time_w, time_b, wq, bq, wk, bk, wv, bv, we, aW, abias):
    from concourse import bass_utils

    n_id = np.asarray(n_id)
    edge_index = np.asarray(edge_index)
    t = np.asarray(t)
    msg = np.asarray(msg, dtype=np.float32)
    num_nodes = memory.shape[0]

    cores, meta = _host_prep(n_id, edge_index, t, msg, last_update)
    nc = _build(meta, num_nodes)

    time_w = np.asarray(time_w, dtype=np.float32)
    time_b = np.asarray(time_b, dtype=np.float32)
    brow = np.concatenate([np.asarray(bq), np.asarray(bk), np.asarray(bv),
                           np.asarray(abias)]).reshape(1, -1).astype(np.float32)
    shared = {
        "memory": np.asarray(memory, dtype=np.float32),
        "static_node_features": np.asarray(static_node_features, dtype=np.float32),
        "last_update": np.asarray(last_update, dtype=np.int32).reshape(-1, 1),
        "enc_wT": np.ascontiguousarray(np.asarray(enc_w, dtype=np.float32).T),
        "wqT": np.ascontiguousarray(np.asarray(wq, dtype=np.float32).T),
        "wkT": np.ascontiguousarray(np.asarray(wk, dtype=np.float32).T),
        "wvT": np.ascontiguousarray(np.asarray(wv, dtype=np.float32).T),
        "weT": np.ascontiguousarray(np.concatenate([
            np.asarray(we, dtype=np.float32).T[0:64],
            np.asarray(we, dtype=np.float32).T[72:128],
            np.asarray(we, dtype=np.float32).T[64:72]], axis=0)),
        "aW": np.asarray(aW, dtype=np.float32),
        "aWT": np.ascontiguousarray(np.asarray(aW, dtype=np.float32).T),
        "brow": brow,
        "encb": np.asarray(enc_b, dtype=np.float32).reshape(1, -1),
        "bqcol": np.asarray(bq, dtype=np.float32).reshape(-1, 1),
        "twrow": time_w.reshape(1, -1),
        "tbq": (time_b * np.float32(INV_2PI) + np.float32(0.25)
                ).astype(np.float32).reshape(-1, 1),
        "tbhp": (time_b + np.float32(HALF_PI)).astype(np.float32).reshape(-1, 1),
        "clhi": (np.float32(PI_CLAMP) - time_b - np.float32(HALF_PI)
                 ).astype(np.float32).reshape(-1, 1),
        "cllo": (-np.float32(PI_CLAMP) - time_b - np.float32(HALF_PI)
                 ).astype(np.float32).reshape(-1, 1),
    }
    in_maps = []
    for c in range(NCORES):
        m = dict(shared)
        m["nid"] = cores[c]["nid"]
        m["msgTa"] = cores[c]["msgTa"]
        m["msgTb"] = cores[c]["msgTb"]
        m["rel"] = cores[c]["rel"]
        m["ld"] = cores[c]["ld"]
        m["kvlo"] = cores[c]["kvlo"]
        m["kvhi"] = cores[c]["kvhi"]
        in_maps.append(m)

    if os.environ.get("KERNEL_SIM", "0") == "1":
        from concourse.bass_interp import MultiCoreSim
        sim = MultiCoreSim(nc, num_cores=NCORES, trace=False,
                           require_finite=False, require_nnan=False)
        cs = list(sim.cores.values())
        for ci, core in enumerate(cs):
            for k, v in in_maps[ci].items():
                core.tensor(k)[:] = v
        sim.simulate(check_with_hw=False, trace_hw=False)

        class R:
            results = [{"out": np.asarray(core.tensor("out"))} for core in cs]
        res = R()
        kernel.last_exec_time_ns = None
        N = meta["N"]
        local_of = meta["local_of"]
        bounds = meta["bounds"]
        out = np.zeros((N, MEM), dtype=np.float32)
        for c in range(NCORES):
            nodes = np.arange(bounds[c], bounds[c + 1])
            out[nodes] = res.results[c]["out"][local_of[nodes]]
        return out

    trace = os.environ.get("KERNEL_TRACE", "0") == "1"
    res = bass_utils.run_bass_kernel_spmd(
        nc, in_maps, core_ids=list(range(NCORES)), trace=trace)
    if trace:
        print("HW exec time:", res.exec_time_ns, "ns")
        kernel.last_exec_time_ns = res.exec_time_ns
        kernel.last_trace = res.instructions_and_trace

    # unshard: core c's rows [local] -> original node id order
    N = meta["N"]
    local_of = meta["local_of"]
    bounds = meta["bounds"]
    out = np.zeros((N, MEM), dtype=np.float32)
    for c in range(NCORES):
        nodes = np.arange(bounds[c], bounds[c + 1])
        out[nodes] = res.results[c]["out"][local_of[nodes]]
    return out
